# revision 1
# baseline (speedup 1.0000x reference)
"""EMA kernel for Trainium2: y[t] = alpha*x[t] + (1-alpha)*y[t-1], y_prev init = x[:, 0].

Sharding: pure data parallel over B=512 rows -> 64 rows/core on 8 cores.
Each core's [64, 65536] block is folded to [128, 32768]: partitions 0..63
hold the first T-half of each row, partitions 64..127 the second T-half,
so all 128 SBUF partitions are busy.

The recurrence runs on the DVE's native tensor_tensor_scan
(state = (data0 * state) + data1, i.e. 0.7*state + alpha*x), chained
across column tiles via initial=prev_tile[:, -1:].  The second T-half's
initial carry is recovered with a HALO-column warm-up scan over the tail
of the first half: (1-alpha)^128 ~= 1.5e-20, far below fp32 resolution,
so the result is exact to fp32.  The alpha pre-scale runs on the Scalar
(ACT) engine so the DVE only does the scan pass.

Engine/issue layout (kernel is DMA-bound; modeled ~96.7us/core vs a
~93.3us wire floor for 32 MB of HBM traffic at ~360 GB/s):
  - input-tile loads issued by the SP sequencer (HWDGE direct-2D)
  - output-tile stores issued by the ACT sequencer (HWDGE) so the two
    directions don't serialize on one issuing sequencer
  - tiny halo/carry loads on gpsimd (SWDGE), off the critical path

Built on bacc.Bacc (not raw bass.Bass): TRN2 instructions fit at most
ONE sync-wait command, and Bacc.compile()'s generate_event_semaphores
pass legalizes Tile's multi-wait instructions by splitting extra waits
into InstEventSemaphore ops.
"""

import numpy as np

ALPHA = 0.3
B, T = 512, 65536
N_CORES = 8
ROWS_PER_CORE = B // N_CORES  # 64
HALF_T = T // 2  # 32768
P = 128
HALO = 128
TILE_COLS = 1024
BUFS = 6

_CACHE: dict = {}


def _build_nc(n_cols: int, tile_cols: int, halo: int, bufs: int = BUFS, end_taper: int = 0, halo_eng: str = 'gpsimd'):
    import concourse.bacc as bacc
    import concourse.mybir as mybir
    from concourse.tile import TileContext

    nc = bacc.Bacc(
        "TRN2", target_bir_lowering=False, debug=False, num_devices=N_CORES
    )
    x = nc.dram_tensor("x", [P, n_cols], mybir.dt.float32, kind="ExternalInput").ap()
    y = nc.dram_tensor("y", [P, n_cols], mybir.dt.float32, kind="ExternalOutput").ap()

    alpha = float(np.float32(ALPHA))
    one_m_alpha = float(np.float32(1.0) - np.float32(ALPHA))
    n_tiles = (n_cols + tile_cols - 1) // tile_cols
    H = P // 2  # 64

    with TileContext(nc) as tc:
        with (
            tc.tile_pool(name="const", bufs=1) as cpool,
            tc.tile_pool(name="xin", bufs=bufs) as xpool,
            tc.tile_pool(name="xscaled", bufs=bufs) as spool,
            tc.tile_pool(name="yout", bufs=bufs) as ypool,
            tc.tile_pool(name="halo", bufs=1) as hpool,
        ):
            const7 = cpool.tile([P, tile_cols], mybir.dt.float32)
            nc.vector.memset(const7[:], one_m_alpha)

            carry = hpool.tile([P, 1], mybir.dt.float32)
            # Partitions 0..63 start the true sequence: initial state = x[:, 0]
            # (reference initializes y_prev to x[:, 0]).
            HALO_DMA = {'gpsimd': nc.gpsimd, 'sync': nc.sync, 'scalar': nc.scalar}[halo_eng].dma_start
            HALO_DMA(carry[0:H, :], x[0:H, 0:1])
            # Partitions 64..127 resume mid-sequence: warm up the state over
            # the last `halo` columns of the first half (same rows, which are
            # partitions 0..63 of this core's input).
            hraw = hpool.tile([P, halo], mybir.dt.float32)
            HALO_DMA(hraw[H:P, :], x[0:H, n_cols - halo : n_cols])
            hs = hpool.tile([P, halo], mybir.dt.float32)
            nc.scalar.mul(hs[H:P, :], hraw[H:P, :], alpha)
            hy = hpool.tile([P, halo], mybir.dt.float32)
            nc.vector.tensor_tensor_scan(
                hy[H:P, :],
                const7[H:P, 0:halo],
                hs[H:P, :],
                0.0,
                mybir.AluOpType.mult,
                mybir.AluOpType.add,
            )
            nc.vector.tensor_copy(carry[H:P, :], hy[H:P, halo - 1 : halo])

            widths = [tile_cols] * n_tiles
            if end_taper and n_cols % tile_cols == 0 and tile_cols % 4 == 0:
                q = tile_cols // 4
                widths = [tile_cols] * (n_tiles - 1) + [2 * q, q, q]
            prev_carry = carry[:, 0:1]
            c_next = 0
            for j, w in enumerate(widths):
                c0 = c_next
                c1 = c0 + w
                c_next = c1
                xt = xpool.tile([P, tile_cols], mybir.dt.float32)
                nc.sync.dma_start(xt[:, 0:w], x[:, c0:c1])
                xs = spool.tile([P, tile_cols], mybir.dt.float32)
                nc.scalar.mul(xs[:, 0:w], xt[:, 0:w], alpha)
                yt = ypool.tile([P, tile_cols], mybir.dt.float32)
                nc.vector.tensor_tensor_scan(
                    yt[:, 0:w],
                    const7[:, 0:w],
                    xs[:, 0:w],
                    prev_carry,
                    mybir.AluOpType.mult,
                    mybir.AluOpType.add,
                )
                nc.scalar.dma_start(y[:, c0:c1], yt[:, 0:w])
                prev_carry = yt[:, w - 1 : w]

    nc.compile()
    return nc


def _get_nc():
    key = (HALF_T, TILE_COLS, HALO)
    if key not in _CACHE:
        _CACHE[key] = _build_nc(*key)
    return _CACHE[key]


def _shard(x: np.ndarray) -> list[dict]:
    in_maps = []
    for c in range(N_CORES):
        rows = x[c * ROWS_PER_CORE : (c + 1) * ROWS_PER_CORE]
        xc = np.concatenate([rows[:, :HALF_T], rows[:, HALF_T:]], axis=0)
        in_maps.append({"x": np.ascontiguousarray(xc)})
    return in_maps


def _unshard(results: list[dict]) -> np.ndarray:
    out = np.empty((B, T), np.float32)
    for c in range(N_CORES):
        yc = results[c]["y"]
        r0 = c * ROWS_PER_CORE
        out[r0 : r0 + ROWS_PER_CORE, :HALF_T] = yc[:ROWS_PER_CORE]
        out[r0 : r0 + ROWS_PER_CORE, HALF_T:] = yc[ROWS_PER_CORE:]
    return out


def kernel(f0_frames: np.ndarray, **kwargs) -> np.ndarray:
    import time

    from concourse.bass_utils import run_bass_kernel_spmd

    x = np.ascontiguousarray(np.asarray(f0_frames), dtype=np.float32)
    assert x.shape == (B, T), x.shape
    nc = _get_nc()
    in_maps = _shard(x)
    # The axon terminal occasionally reports NRT_EXEC_UNIT_UNRECOVERABLE when
    # a dispatch lands while the device is still recycling from a previous
    # process; a backend reset + retry after a pause recovers it.
    last_err = None
    for attempt in range(3):
        if attempt:
            time.sleep(30)
            try:
                from jax.extend.backend import clear_backends

                clear_backends()
            except Exception:
                pass
        try:
            res = run_bass_kernel_spmd(nc, in_maps, core_ids=list(range(N_CORES)))
            return _unshard(res.results)
        except Exception as e:  # noqa: BLE001 - retry transient device errors
            last_err = e
    raise last_err

